# revision 42
# baseline (speedup 1.0000x reference)
"""AreaAttention Trainium2 kernel: B=8 data-parallel over 8 NeuronCores.

Reference computation (per sample, C=128 channels, N=H*W=4096 pixels):
    q = Wq@x + bq                    ('oc,bcn->bno' proper matmul)
    k = x * colsum(Wk) + bk          ('oc,bcn->bcn' keeps c: per-channel scale!)
    v = Wv@x + bv                    ('oc,bcn->bno')
    out = x + softmax(q^T k / sqrt(C)) @ v^T

Per-core design (one sample per core, no collectives):
  - q16/k16 stored [c, n] fp16; v16 stored [m, c] fp16 (PSUM accumulates fp32).
  - Scores computed TRANSPOSED: sT[m, n] = k_chunk^T @ q  (free dim 512).
  - exp((s*scale)+E) with global shift E=+1 (cancels in softmax). Most pairs
    on ScalarE (ACT); DVE_PAIRS per block are computed on the Vector engine
    via a Schraudolph bit-trick: fp16_bits = round(A*(s*scale+E)+B) emitted
    by one tensor_scalar into a uint16 view (rms rel err ~1.8%, verified on
    HW). ACT and DVE exps run CONCURRENTLY on different PSUM banks, lifting
    the per-block exp throughput above either engine alone.
  - PV: out[c, n] += v_chunk^T @ expS  -> output directly in [c, n] layout.
  - Softmax denominator: 2 fp16 partial-sum chains over chunk-pairs, ones-
    matmul reduces partitions AND broadcasts row-sums; reciprocal via the
    single-instruction reciprocal_approx_fast (fp32, ~51 ULP), multiply,
    +residual.
  - Setup offload: xf16 casts / k16 scale / residual adds run on the idle
    GpSimd engine; q-projections are spread across blocks (q slice s+1 is
    produced during block s) to unclog DVE in block 0.
"""
import numpy as np

C = 128
N = 4096          # 64*64
NB = 1024         # n-block span
NBLK = N // NB    # 4
MCH = N // C      # 32 m-chunks
NPAIR = MCH // 2  # 16 chunk-pairs per block
SCALE = 1.0 / np.sqrt(np.float32(C))
ESHIFT = 1.0      # global softmax shift: es = exp(s*scale + E); cancels.

# Schraudolph fp16 exp: bits = round(A16*(arg) + B16); bitcast -> fp16 ~ e^arg
A16 = 1477.319722
B16 = 15300.5
DVE_MULT = float(A16 * SCALE)
DVE_ADD = float(A16 * ESHIFT + B16)
# pairs computed on DVE instead of ACT, per block (avoid the hoisted pairs
# {0,1,2} and block-0's setup jps; block 0's DVE is busier with v16/qproj)
DVE_PAIRS = {0: (5, 12), 1: (6, 12), 2: (6, 12), 3: (6, 12)}
# the very last pair: chunk u=0 on ACT, u=1 on DVE, so both exps finish
# together and the final tail starts as early as possible
SPLIT_PAIR = (3, 15)

_cache = {}


def _build_nc():
    import concourse.tile as tile
    from concourse import bacc, mybir

    f32 = mybir.dt.float32
    f16 = mybir.dt.float16
    u16 = mybir.dt.uint16
    ADD = mybir.AluOpType.add
    MUL = mybir.AluOpType.mult
    EXP = mybir.ActivationFunctionType.Exp

    nc = bacc.Bacc("TRN2", target_bir_lowering=False)

    x_d = nc.dram_tensor("x", [C, N], f32, kind="ExternalInput")
    # packed weights: one tiny DMA each instead of six serialized ones
    wqv16_d = nc.dram_tensor("wqv16", [C, 2 * C], f16, kind="ExternalInput")
    smalls_d = nc.dram_tensor("smalls", [C, 4], f32, kind="ExternalInput")
    out_d = nc.dram_tensor("out", [C, N], f32, kind="ExternalOutput")

    with tile.TileContext(nc) as tc:
        with tc.tile_pool(name="big", bufs=1) as big, \
             tc.tile_pool(name="small", bufs=1) as small, \
             tc.tile_pool(name="es_pool", bufs=8) as es_pool, \
             tc.tile_pool(name="p_pool", bufs=6) as p_pool, \
             tc.tile_pool(name="work", bufs=2) as work, \
             tc.tile_pool(name="ps_sc", bufs=3, space="PSUM") as ps_sc, \
             tc.tile_pool(name="ps_pv", bufs=1, space="PSUM") as ps_pv:

            xfb = big.tile([C, N], f32, tag="xfb")      # x, then x + bv (residual)
            xf16 = big.tile([C, N], f16, tag="xf16")    # x fp16 (q/v proj, k build)
            q16 = big.tile([C, N], f16, tag="q16")
            k16 = big.tile([C, N], f16, tag="k16")
            v16 = big.tile([C, N], f16, tag="v16")      # chunk j at cols [128j,128j+128) = v[m, c]

            wqv16 = small.tile([C, 2 * C], f16, tag="wqv16")
            smalls = small.tile([C, 4], f32, tag="smalls")
            ebias = small.tile([C, 1], f32, tag="ebias")
            ones16 = small.tile([C, C], f16, tag="ones16")
            wqt16 = wqv16[:, 0:C]
            wvt16 = wqv16[:, C:2 * C]
            wks = smalls[:, 0:1]
            bk = smalls[:, 1:2]
            bq = smalls[:, 2:3]
            bv = smalls[:, 3:4]

            # Per-queue DMA bandwidth is ~50GB/s, so spread the loads: slice 0
            # split across sync+vector queues (it gates the first QK), the
            # small weights FIRST on the scalar queue (they gate qproj/k16 and
            # would otherwise queue behind megabytes of x), bulk x on gpsimd.
            nc.scalar.dma_start(smalls[:], smalls_d[:])
            nc.scalar.dma_start(wqv16[:], wqv16_d[:])
            nc.sync.dma_start(xfb[:, 0:256], x_d[:, 0:256])
            nc.scalar.dma_start(xfb[:, 256:512], x_d[:, 256:512])
            nc.sync.dma_start(xfb[:, 512:768], x_d[:, 512:768])
            nc.scalar.dma_start(xfb[:, 768:NB], x_d[:, 768:NB])
            nc.sync.dma_start(xfb[:, NB:2 * NB], x_d[:, NB:2 * NB])
            nc.gpsimd.dma_start(xfb[:, 2 * NB:3 * NB], x_d[:, 2 * NB:3 * NB])
            nc.sync.dma_start(xfb[:, 3 * NB:4 * NB], x_d[:, 3 * NB:4 * NB])
            nc.vector.memset(ebias[:], ESHIFT)
            nc.vector.memset(ones16[:], 1.0)

            def cast_k(s):
                """xf16 cast + k16 build for one slice (DVE)."""
                sl = slice(s * NB, (s + 1) * NB)
                nc.vector.tensor_copy(xf16[:, sl], xfb[:, sl])
                nc.vector.tensor_scalar(k16[:, sl], xf16[:, sl], wks, bk,
                                        op0=MUL, op1=ADD)

            def qproj(s):
                """q16 for one 1024-col slice (2 matmuls + DVE bias add)."""
                sl = slice(s * NB, (s + 1) * NB)
                ps = ps_sc.tile([C, NB], f32, tag="sc", name=f"qps{s}")
                for h in range(2):
                    hsl = slice(s * NB + h * 512, s * NB + (h + 1) * 512)
                    nc.tensor.matmul(ps[:, h * 512:(h + 1) * 512], wqt16,
                                     xf16[:, hsl], start=True, stop=True)
                nc.vector.tensor_scalar(q16[:, sl], ps[:], bq, None, op0=ADD)

            def setup_v(s):
                """v16 chunks for one 1024-col slice (8 chunks, one psum tile)."""
                sl = slice(s * NB, (s + 1) * NB)
                psv = ps_sc.tile([C, NB], f32, tag="sc", name=f"vps{s}")
                for t in range(8):
                    j = s * 8 + t
                    nc.tensor.matmul(psv[:, t * C:(t + 1) * C],
                                     xf16[:, j * C:(j + 1) * C], wvt16,
                                     start=True, stop=True)
                nc.vector.tensor_copy(v16[:, sl], psv[:])

            def resid_slice(s):
                sl = slice(s * NB, (s + 1) * NB)
                nc.vector.tensor_scalar(xfb[:, sl], xfb[:, sl], bv, None, op0=ADD)

            def tail(nb, pv, chains, last_es2):
                n0 = nb * NB
                last = nb == NBLK - 1
                # row-sum over partitions; the ones[128,128] stationary operand
                # broadcasts the sum to all partitions. Sources: the two chains
                # (ready one pair early) + the final pair's es2 directly.
                # per-half rs tiles: half 1's rowsum matmuls must not carry a
                # tile-level WAR dependency on half 0's reciprocal read
                if last:
                    rs_t = [ps_sc.tile([C, 512], f32, tag="sc", name=f"rs{nb}_{h}")
                            for h in range(2)]
                else:
                    rs1 = ps_sc.tile([C, NB], f32, tag="sc", name=f"rs{nb}")
                    rs_t = [rs1[:, 0:512], rs1[:, 512:NB]]
                srcs = [(chains[0], slice(0, NB)), (chains[1], slice(0, NB))]
                if last_es2 is not None:
                    srcs.append((last_es2, slice(0, NB)))
                # single-instruction approximate reciprocal (fp32, ~51 ULP),
                # then normalize + residual and DMA out; in the final block
                # the rowsums are inside the half loop so half 0's normalize
                # overlaps half 1's rowsum matmuls.
                rb = work.tile([C, NB], f32, tag="rb", name=f"rb{nb}")
                ep1 = work.tile([C, NB], f32, tag="ep1", name=f"ep{nb}")
                ost = work.tile([C, NB], f32, tag="ost", name=f"ost{nb}")
                halves = (slice(0, 512), slice(512, NB)) if last else (slice(0, NB),)
                for hs in halves:
                    for h in (0, 1) if hs.stop - hs.start == NB else (hs.start // 512,):
                        hsl = slice(h * 512, (h + 1) * 512)
                        for ci, (src, _) in enumerate(srcs):
                            dst = rs_t[h][:, 0:512] if last else rs1[:, hsl]
                            nc.tensor.matmul(dst, ones16[:], src[:, hsl],
                                             start=(ci == 0), stop=False)
                            nc.tensor.matmul(dst, ones16[:],
                                             src[:, NB + h * 512:NB + (h + 1) * 512],
                                             start=False, stop=(ci == len(srcs) - 1))
                    osl = slice(n0 + hs.start, n0 + hs.stop)
                    if last:
                        rsrc = rs_t[hs.start // 512][:, 0:512]
                    else:
                        rsrc = rs1[:, hs]
                    nc.vector.reciprocal_approx_fast(out=rb[:, hs], in_=rsrc)
                    nc.vector.tensor_tensor(ep1[:, hs], pv[:, hs], rb[:, hs], op=MUL)
                    if last:
                        # quarter-grain ost + DMA over three queues so the
                        # final transfer starts (and is) small
                        qw = (hs.stop - hs.start) // 2
                        engs = (nc.sync, nc.scalar) if hs.start == 0 \
                            else (nc.gpsimd, nc.sync)
                        for qi, eng in enumerate(engs):
                            a = hs.start + qi * qw
                            qsl = slice(a, a + qw)
                            nc.vector.tensor_tensor(ost[:, qsl], ep1[:, qsl],
                                                    xfb[:, n0 + a:n0 + a + qw],
                                                    op=ADD)
                            eng.dma_start(out_d[:, n0 + a:n0 + a + qw],
                                          ost[:, qsl])
                    else:
                        nc.vector.tensor_tensor(ost[:, hs], ep1[:, hs],
                                                xfb[:, osl], op=ADD)
                        nc.sync.dma_start(out_d[:, osl], ost[:, hs])

            # slice-0 setup at half granularity so the first QK/exp is not
            # gated on the second x-half DMA; later slices' casts are
            # interleaved into block 0 (setup_v needs xf16 of every slice).
            for h in range(2):
                hsl = slice(h * 512, (h + 1) * 512)
                qps0 = ps_sc.tile([C, 512], f32, tag="sc", name=f"qps0_{h}")
                nc.vector.tensor_copy(xf16[:, hsl], xfb[:, hsl])
                nc.vector.tensor_scalar(k16[:, hsl], xf16[:, hsl], wks, bk,
                                        op0=MUL, op1=ADD)
                nc.tensor.matmul(qps0[:, 0:512], wqt16, xf16[:, hsl],
                                 start=True, stop=True)
                nc.vector.tensor_scalar(q16[:, hsl], qps0[:, 0:512], bq, None,
                                        op0=ADD)

            def emit_qk_exp(nb, jp):
                """QK matmuls + exp for one chunk-pair; returns the es2 tile.

                ACT pairs: scalar.activation Exp (exact). DVE_PAIRS: one
                vector.tensor_scalar per chunk writing Schraudolph fp16 bits
                through a uint16 view (runs concurrently with ACT pairs).
                """
                n0 = nb * NB
                es2 = es_pool.tile([C, 2 * NB], f16, tag="es", name=f"es{nb}_{jp}")
                for u in range(2):
                    on_dve = (jp in DVE_PAIRS[nb]) or ((nb, jp) == SPLIT_PAIR and u == 1)
                    j = 2 * jp + u
                    ksl = slice(j * C, (j + 1) * C)
                    sc = ps_sc.tile([C, NB], f32, tag="sc", name=f"sc{nb}_{j}")
                    for h in range(2):
                        nc.tensor.matmul(sc[:, h * 512:(h + 1) * 512],
                                         k16[:, ksl],
                                         q16[:, n0 + h * 512:n0 + (h + 1) * 512],
                                         start=True, stop=True)
                    usl = slice(u * NB, (u + 1) * NB)
                    if on_dve:
                        nc.vector.tensor_scalar(es2[:, usl].bitcast(u16), sc[:],
                                                DVE_MULT, DVE_ADD,
                                                op0=MUL, op1=ADD)
                    elif nb == 0 and jp == 0:
                        # very first pair: per-512 activations so the first
                        # exp starts before the second QK half lands
                        for h in range(2):
                            ql = slice(u * NB + h * 512, u * NB + (h + 1) * 512)
                            nc.scalar.activation(es2[:, ql],
                                                 sc[:, h * 512:(h + 1) * 512],
                                                 EXP, bias=ebias[:],
                                                 scale=float(SCALE))
                    else:
                        nc.scalar.activation(es2[:, usl], sc[:], EXP,
                                             bias=ebias[:], scale=float(SCALE))
                return es2

            def emit_pv(nb, jp, pv, es2):
                for u in range(2):
                    j = 2 * jp + u
                    ksl = slice(j * C, (j + 1) * C)
                    for h in range(2):
                        nc.tensor.matmul(pv[:, h * 512:(h + 1) * 512],
                                         v16[:, ksl],
                                         es2[:, u * NB + h * 512:u * NB + (h + 1) * 512],
                                         start=(j == 0), stop=(j == MCH - 1))

            NHOIST = 3  # next-block QK/exp pairs emitted ahead of each tail
            hoisted = {}
            # pre-fill block 0's first pairs so ACT starts saturated before
            # the vproj/qproj setup bursts take PE slots
            for hj in range(NHOIST):
                hoisted[(0, hj)] = emit_qk_exp(0, hj)
            for nb in range(NBLK):
                pv = ps_pv.tile([C, NB], f32, tag="pv", name=f"pv{nb}")
                chains = [p_pool.tile([C, 2 * NB], f16, tag="pacc", name=f"pacc{nb}_{i}")
                          for i in range(2)]
                pend = None  # deferred chain op: DVE exps jump ahead of it

                def flush_chain():
                    nonlocal pend
                    if pend is None:
                        return
                    jq, es = pend
                    pend = None
                    if jq < 2:
                        nc.vector.tensor_copy(chains[jq % 2][:], es[:])
                    else:
                        nc.vector.tensor_tensor(chains[jq % 2][:],
                                                chains[jq % 2][:], es[:], op=ADD)

                for jp in range(NPAIR):
                    if nb == 0 and jp in (3, 7, 11):
                        s = (jp + 1) // 4       # slices 1, 2, 3
                        cast_k(s)
                        setup_v(s)
                    if nb == 0 and jp in (4, 8, 12):
                        qproj(jp // 4)          # all q slices made in block 0
                        # (block 0 is ACT-bound; PE/DVE have slack here)
                    if jp == 9:
                        resid_slice(nb)         # xfb += bv for this block
                    es2 = hoisted.pop((nb, jp), None)
                    if es2 is None:
                        es2 = emit_qk_exp(nb, jp)
                    flush_chain()               # previous pair's chain op
                    if nb == 0 and jp == 0:
                        setup_v(0)
                    # hoist the NEXT block's first QK/exp pairs ahead of this
                    # block's final PVs + tail so ACT never waits at the boundary
                    if jp == NPAIR - 1 and nb < NBLK - 1:
                        for hj in range(NHOIST):
                            hoisted[(nb + 1, hj)] = emit_qk_exp(nb + 1, hj)
                    emit_pv(nb, jp, pv, es2)
                    # denominator: 2 interleaved fp16 chains over chunk-pairs,
                    # deferred one pair so a DVE-pair exp runs before it. In
                    # the FINAL block the last pair skips the chain -- its
                    # contribution goes straight into the rowsum matmuls.
                    if jp == NPAIR - 1 and nb == NBLK - 1:
                        last_es2 = es2
                    else:
                        pend = (jp, es2)

                flush_chain()
                tail(nb, pv, chains, last_es2 if nb == NBLK - 1 else None)

    nc.finalize()
    return nc


def _get_nc():
    if "nc" not in _cache:
        _cache["nc"] = _build_nc()
    return _cache["nc"]


def make_in_maps(x, Wq, bq, Wk, bk, Wv, bv):
    x = np.asarray(x, dtype=np.float32)
    B = x.shape[0]
    wqt16 = np.asarray(Wq, np.float32).T.astype(np.float16)
    wvt16 = np.asarray(Wv, np.float32).T.astype(np.float16)
    wqv16 = np.ascontiguousarray(np.concatenate([wqt16, wvt16], axis=1))
    wks = np.asarray(Wk, np.float32).sum(axis=0)
    smalls = np.ascontiguousarray(np.stack(
        [wks, np.asarray(bk, np.float32), np.asarray(bq, np.float32),
         np.asarray(bv, np.float32)], axis=1).astype(np.float32))
    in_maps = []
    for i in range(B):
        in_maps.append({
            "x": np.ascontiguousarray(x[i].reshape(C, N)),
            "wqv16": wqv16, "smalls": smalls,
        })
    return in_maps


def kernel(x, Wq, bq, Wk, bk, Wv, bv, _trace=False, _tmpdir=None):
    from concourse.bass_utils import run_bass_kernel_spmd

    x = np.asarray(x, dtype=np.float32)
    B, c, H, W = x.shape
    assert (c, H * W) == (C, N), (c, H, W)
    in_maps = make_in_maps(x, Wq, bq, Wk, bk, Wv, bv)
    nc = _get_nc()
    res = run_bass_kernel_spmd(nc, in_maps, core_ids=list(range(B)),
                               trace=_trace, tmpdir=_tmpdir)
    out = np.stack([res.results[i]["out"].reshape(C, H, W) for i in range(B)])
    if _trace:
        _cache["last_result"] = res
    return out.astype(np.float32)


# revision 44
# speedup vs baseline: 1.1948x; 1.1948x over previous
"""AreaAttention Trainium2 kernel: B=8 data-parallel over 8 NeuronCores.

Reference computation (per sample, C=128 channels, N=H*W=4096 pixels):
    q = Wq@x + bq                    ('oc,bcn->bno' proper matmul)
    k = x * colsum(Wk) + bk          ('oc,bcn->bcn' keeps c: per-channel scale!)
    v = Wv@x + bv                    ('oc,bcn->bno')
    out = x + softmax(q^T k / sqrt(C)) @ v^T

Per-core design (one sample per core, no collectives):
  - q16/k16 stored [c, n] fp16; v16 stored [m, c] fp16 (PSUM accumulates fp32).
  - Scores computed TRANSPOSED: sT[m, n] = k_chunk^T @ q  (free dim 512).
  - exp((s*scale)+E) with global shift E=+1 (cancels in softmax). Most pairs
    on ScalarE (ACT); DVE_PAIRS per block are computed on the Vector engine
    via a Schraudolph bit-trick: fp16_bits = round(A*(s*scale+E)+B) emitted
    by one tensor_scalar into a uint16 view (rms rel err ~1.8%, verified on
    HW). ACT and DVE exps run CONCURRENTLY on different PSUM banks, lifting
    the per-block exp throughput above either engine alone.
  - PV: out[c, n] += v_chunk^T @ expS  -> output directly in [c, n] layout.
  - Softmax denominator: 2 fp16 partial-sum chains over chunk-pairs, ones-
    matmul reduces partitions AND broadcasts row-sums; reciprocal via the
    single-instruction reciprocal_approx_fast (fp32, ~51 ULP), multiply,
    +residual.
  - Setup offload: xf16 casts / k16 scale / residual adds run on the idle
    GpSimd engine; q-projections are spread across blocks (q slice s+1 is
    produced during block s) to unclog DVE in block 0.
"""
import numpy as np

C = 128
N = 4096          # 64*64
NB = 1024         # n-block span
NBLK = N // NB    # 4
MCH = N // C      # 32 m-chunks
NPAIR = MCH // 2  # 16 chunk-pairs per block
SCALE = 1.0 / np.sqrt(np.float32(C))
ESHIFT = 1.0      # global softmax shift: es = exp(s*scale + E); cancels.

# Schraudolph fp16 exp: bits = round(A16*(arg) + B16); bitcast -> fp16 ~ e^arg
A16 = 1477.319722
B16 = 15300.5
DVE_MULT = float(A16 * SCALE)
DVE_ADD = float(A16 * ESHIFT + B16)
# pairs computed on DVE instead of ACT, per block (avoid the hoisted pairs
# {0,1,2} and block-0's setup jps; block 0's DVE is busier with v16/qproj)
DVE_PAIRS = {0: (5, 12), 1: (6, 12), 2: (6, 12), 3: (6, 12)}
# the very last pair: chunk u=0 on ACT, u=1 on DVE, so both exps finish
# together and the final tail starts as early as possible
SPLIT_PAIR = (3, 15)

_cache = {}


def _build_nc():
    import concourse.tile as tile
    from concourse import bacc, mybir

    f32 = mybir.dt.float32
    f16 = mybir.dt.float16
    u16 = mybir.dt.uint16
    ADD = mybir.AluOpType.add
    MUL = mybir.AluOpType.mult
    EXP = mybir.ActivationFunctionType.Exp

    nc = bacc.Bacc("TRN2", target_bir_lowering=False)

    x_d = nc.dram_tensor("x", [C, N], f32, kind="ExternalInput")
    # packed weights: one tiny DMA each instead of six serialized ones
    wqv16_d = nc.dram_tensor("wqv16", [C, 2 * C], f16, kind="ExternalInput")
    smalls_d = nc.dram_tensor("smalls", [C, 4], f32, kind="ExternalInput")
    out_d = nc.dram_tensor("out", [C, N], f32, kind="ExternalOutput")

    with tile.TileContext(nc) as tc:
        with tc.tile_pool(name="big", bufs=1) as big, \
             tc.tile_pool(name="small", bufs=1) as small, \
             tc.tile_pool(name="es_pool", bufs=8) as es_pool, \
             tc.tile_pool(name="p_pool", bufs=6) as p_pool, \
             tc.tile_pool(name="work", bufs=2) as work, \
             tc.tile_pool(name="ps_sc", bufs=3, space="PSUM") as ps_sc, \
             tc.tile_pool(name="ps_pv", bufs=1, space="PSUM") as ps_pv:

            xfb = big.tile([C, N], f32, tag="xfb")      # x, then x + bv (residual)
            xf16 = big.tile([C, N], f16, tag="xf16")    # x fp16 (q/v proj, k build)
            q16 = big.tile([C, N], f16, tag="q16")
            k16 = big.tile([C, N], f16, tag="k16")
            v16 = big.tile([C, N], f16, tag="v16")      # chunk j at cols [128j,128j+128) = v[m, c]

            wqv16 = small.tile([C, 2 * C], f16, tag="wqv16")
            smalls = small.tile([C, 4], f32, tag="smalls")
            ebias = small.tile([C, 1], f32, tag="ebias")
            ones16 = small.tile([C, C], f16, tag="ones16")
            wqt16 = wqv16[:, 0:C]
            wvt16 = wqv16[:, C:2 * C]
            wks = smalls[:, 0:1]
            bk = smalls[:, 1:2]
            bq = smalls[:, 2:3]
            bv = smalls[:, 3:4]

            # Per-queue DMA bandwidth is ~50GB/s, so spread the loads: slice 0
            # split across sync+vector queues (it gates the first QK), the
            # small weights FIRST on the scalar queue (they gate qproj/k16 and
            # would otherwise queue behind megabytes of x), bulk x on gpsimd.
            nc.scalar.dma_start(smalls[:], smalls_d[:])
            nc.scalar.dma_start(wqv16[:], wqv16_d[:])
            nc.sync.dma_start(xfb[:, 0:512], x_d[:, 0:512])
            nc.scalar.dma_start(xfb[:, 512:NB], x_d[:, 512:NB])
            nc.sync.dma_start(xfb[:, NB:2 * NB], x_d[:, NB:2 * NB])
            nc.gpsimd.dma_start(xfb[:, 2 * NB:3 * NB], x_d[:, 2 * NB:3 * NB])
            nc.sync.dma_start(xfb[:, 3 * NB:4 * NB], x_d[:, 3 * NB:4 * NB])
            nc.vector.memset(ebias[:], ESHIFT)
            nc.vector.memset(ones16[:], 1.0)

            def cast_k(s):
                """xf16 cast + k16 build for one slice (DVE)."""
                sl = slice(s * NB, (s + 1) * NB)
                nc.vector.tensor_copy(xf16[:, sl], xfb[:, sl])
                nc.vector.tensor_scalar(k16[:, sl], xf16[:, sl], wks, bk,
                                        op0=MUL, op1=ADD)

            def qproj(s):
                """q16 for one 1024-col slice (2 matmuls + DVE bias add)."""
                sl = slice(s * NB, (s + 1) * NB)
                ps = ps_sc.tile([C, NB], f32, tag="sc", name=f"qps{s}")
                for h in range(2):
                    hsl = slice(s * NB + h * 512, s * NB + (h + 1) * 512)
                    nc.tensor.matmul(ps[:, h * 512:(h + 1) * 512], wqt16,
                                     xf16[:, hsl], start=True, stop=True)
                nc.vector.tensor_scalar(q16[:, sl], ps[:], bq, None, op0=ADD)

            def setup_v(s):
                """v16 chunks for one 1024-col slice (8 chunks, one psum tile)."""
                sl = slice(s * NB, (s + 1) * NB)
                psv = ps_sc.tile([C, NB], f32, tag="sc", name=f"vps{s}")
                for t in range(8):
                    j = s * 8 + t
                    nc.tensor.matmul(psv[:, t * C:(t + 1) * C],
                                     xf16[:, j * C:(j + 1) * C], wvt16,
                                     start=True, stop=True)
                nc.vector.tensor_copy(v16[:, sl], psv[:])

            def resid_slice(s):
                sl = slice(s * NB, (s + 1) * NB)
                nc.vector.tensor_scalar(xfb[:, sl], xfb[:, sl], bv, None, op0=ADD)

            def tail(nb, pv, chains, last_es2):
                n0 = nb * NB
                last = nb == NBLK - 1
                # row-sum over partitions; the ones[128,128] stationary operand
                # broadcasts the sum to all partitions. Sources: the two chains
                # (ready one pair early) + the final pair's es2 directly.
                # per-half rs tiles: half 1's rowsum matmuls must not carry a
                # tile-level WAR dependency on half 0's reciprocal read
                if last:
                    rs_t = [ps_sc.tile([C, 512], f32, tag="sc", name=f"rs{nb}_{h}")
                            for h in range(2)]
                else:
                    rs1 = ps_sc.tile([C, NB], f32, tag="sc", name=f"rs{nb}")
                    rs_t = [rs1[:, 0:512], rs1[:, 512:NB]]
                srcs = [(chains[0], slice(0, NB)), (chains[1], slice(0, NB))]
                if last_es2 is not None:
                    srcs.append((last_es2, slice(0, NB)))
                # single-instruction approximate reciprocal (fp32, ~51 ULP),
                # then normalize + residual and DMA out; in the final block
                # the rowsums are inside the half loop so half 0's normalize
                # overlaps half 1's rowsum matmuls.
                rb = work.tile([C, NB], f32, tag="rb", name=f"rb{nb}")
                ep1 = work.tile([C, NB], f32, tag="ep1", name=f"ep{nb}")
                ost = work.tile([C, NB], f32, tag="ost", name=f"ost{nb}")
                halves = (slice(0, 512), slice(512, NB)) if last else (slice(0, NB),)
                for hs in halves:
                    for h in (0, 1) if hs.stop - hs.start == NB else (hs.start // 512,):
                        hsl = slice(h * 512, (h + 1) * 512)
                        for ci, (src, _) in enumerate(srcs):
                            dst = rs_t[h][:, 0:512] if last else rs1[:, hsl]
                            nc.tensor.matmul(dst, ones16[:], src[:, hsl],
                                             start=(ci == 0), stop=False)
                            nc.tensor.matmul(dst, ones16[:],
                                             src[:, NB + h * 512:NB + (h + 1) * 512],
                                             start=False, stop=(ci == len(srcs) - 1))
                    osl = slice(n0 + hs.start, n0 + hs.stop)
                    if last:
                        rsrc = rs_t[hs.start // 512][:, 0:512]
                    else:
                        rsrc = rs1[:, hs]
                    nc.vector.reciprocal_approx_fast(out=rb[:, hs], in_=rsrc)
                    nc.vector.tensor_tensor(ep1[:, hs], pv[:, hs], rb[:, hs], op=MUL)
                    nc.vector.tensor_tensor(ost[:, hs], ep1[:, hs], xfb[:, osl], op=ADD)
                    if last:
                        # split across two queues so the final DMA is small
                        mid = (hs.start + hs.stop) // 2
                        nc.sync.dma_start(out_d[:, n0 + hs.start:n0 + mid],
                                          ost[:, hs.start:mid])
                        nc.scalar.dma_start(out_d[:, n0 + mid:n0 + hs.stop],
                                            ost[:, mid:hs.stop])
                    else:
                        nc.sync.dma_start(out_d[:, osl], ost[:, hs])

            # slice-0 setup at half granularity so the first QK/exp is not
            # gated on the second x-half DMA; later slices' casts are
            # interleaved into block 0 (setup_v needs xf16 of every slice).
            for h in range(2):
                hsl = slice(h * 512, (h + 1) * 512)
                qps0 = ps_sc.tile([C, 512], f32, tag="sc", name=f"qps0_{h}")
                nc.vector.tensor_copy(xf16[:, hsl], xfb[:, hsl])
                nc.vector.tensor_scalar(k16[:, hsl], xf16[:, hsl], wks, bk,
                                        op0=MUL, op1=ADD)
                nc.tensor.matmul(qps0[:, 0:512], wqt16, xf16[:, hsl],
                                 start=True, stop=True)
                nc.vector.tensor_scalar(q16[:, hsl], qps0[:, 0:512], bq, None,
                                        op0=ADD)

            def emit_qk_exp(nb, jp):
                """QK matmuls + exp for one chunk-pair; returns the es2 tile.

                ACT pairs: scalar.activation Exp (exact). DVE_PAIRS: one
                vector.tensor_scalar per chunk writing Schraudolph fp16 bits
                through a uint16 view (runs concurrently with ACT pairs).
                """
                n0 = nb * NB
                es2 = es_pool.tile([C, 2 * NB], f16, tag="es", name=f"es{nb}_{jp}")
                for u in range(2):
                    on_dve = (jp in DVE_PAIRS[nb]) or ((nb, jp) == SPLIT_PAIR and u == 1)
                    j = 2 * jp + u
                    ksl = slice(j * C, (j + 1) * C)
                    sc = ps_sc.tile([C, NB], f32, tag="sc", name=f"sc{nb}_{j}")
                    for h in range(2):
                        nc.tensor.matmul(sc[:, h * 512:(h + 1) * 512],
                                         k16[:, ksl],
                                         q16[:, n0 + h * 512:n0 + (h + 1) * 512],
                                         start=True, stop=True)
                    usl = slice(u * NB, (u + 1) * NB)
                    if on_dve:
                        nc.vector.tensor_scalar(es2[:, usl].bitcast(u16), sc[:],
                                                DVE_MULT, DVE_ADD,
                                                op0=MUL, op1=ADD)
                    elif nb == 0 and jp == 0:
                        # very first pair: per-512 activations so the first
                        # exp starts before the second QK half lands
                        for h in range(2):
                            ql = slice(u * NB + h * 512, u * NB + (h + 1) * 512)
                            nc.scalar.activation(es2[:, ql],
                                                 sc[:, h * 512:(h + 1) * 512],
                                                 EXP, bias=ebias[:],
                                                 scale=float(SCALE))
                    else:
                        nc.scalar.activation(es2[:, usl], sc[:], EXP,
                                             bias=ebias[:], scale=float(SCALE))
                return es2

            def emit_pv(nb, jp, pv, es2):
                for u in range(2):
                    j = 2 * jp + u
                    ksl = slice(j * C, (j + 1) * C)
                    for h in range(2):
                        nc.tensor.matmul(pv[:, h * 512:(h + 1) * 512],
                                         v16[:, ksl],
                                         es2[:, u * NB + h * 512:u * NB + (h + 1) * 512],
                                         start=(j == 0), stop=(j == MCH - 1))

            NHOIST = 3  # next-block QK/exp pairs emitted ahead of each tail
            hoisted = {}
            # pre-fill block 0's first pairs so ACT starts saturated before
            # the vproj/qproj setup bursts take PE slots
            for hj in range(NHOIST):
                hoisted[(0, hj)] = emit_qk_exp(0, hj)
            for nb in range(NBLK):
                pv = ps_pv.tile([C, NB], f32, tag="pv", name=f"pv{nb}")
                chains = [p_pool.tile([C, 2 * NB], f16, tag="pacc", name=f"pacc{nb}_{i}")
                          for i in range(2)]
                pend = None  # deferred chain op: DVE exps jump ahead of it

                def flush_chain():
                    nonlocal pend
                    if pend is None:
                        return
                    jq, es = pend
                    pend = None
                    if jq < 2:
                        nc.vector.tensor_copy(chains[jq % 2][:], es[:])
                    else:
                        nc.vector.tensor_tensor(chains[jq % 2][:],
                                                chains[jq % 2][:], es[:], op=ADD)

                for jp in range(NPAIR):
                    if nb == 0 and jp in (3, 7, 11):
                        s = (jp + 1) // 4       # slices 1, 2, 3
                        cast_k(s)
                        setup_v(s)
                    if nb == 0 and jp in (4, 8, 12):
                        qproj(jp // 4)          # all q slices made in block 0
                        # (block 0 is ACT-bound; PE/DVE have slack here)
                    if jp == 9:
                        resid_slice(nb)         # xfb += bv for this block
                    es2 = hoisted.pop((nb, jp), None)
                    if es2 is None:
                        es2 = emit_qk_exp(nb, jp)
                    flush_chain()               # previous pair's chain op
                    if nb == 0 and jp == 0:
                        setup_v(0)
                    # hoist the NEXT block's first QK/exp pairs ahead of this
                    # block's final PVs + tail so ACT never waits at the boundary
                    if jp == NPAIR - 1 and nb < NBLK - 1:
                        for hj in range(NHOIST):
                            hoisted[(nb + 1, hj)] = emit_qk_exp(nb + 1, hj)
                    emit_pv(nb, jp, pv, es2)
                    # denominator: 2 interleaved fp16 chains over chunk-pairs,
                    # deferred one pair so a DVE-pair exp runs before it. In
                    # the FINAL block the last pair skips the chain -- its
                    # contribution goes straight into the rowsum matmuls.
                    if jp == NPAIR - 1 and nb == NBLK - 1:
                        last_es2 = es2
                    else:
                        pend = (jp, es2)

                flush_chain()
                tail(nb, pv, chains, last_es2 if nb == NBLK - 1 else None)

    nc.finalize()
    return nc


def _get_nc():
    if "nc" not in _cache:
        _cache["nc"] = _build_nc()
    return _cache["nc"]


def make_in_maps(x, Wq, bq, Wk, bk, Wv, bv):
    x = np.asarray(x, dtype=np.float32)
    B = x.shape[0]
    wqt16 = np.asarray(Wq, np.float32).T.astype(np.float16)
    wvt16 = np.asarray(Wv, np.float32).T.astype(np.float16)
    wqv16 = np.ascontiguousarray(np.concatenate([wqt16, wvt16], axis=1))
    wks = np.asarray(Wk, np.float32).sum(axis=0)
    smalls = np.ascontiguousarray(np.stack(
        [wks, np.asarray(bk, np.float32), np.asarray(bq, np.float32),
         np.asarray(bv, np.float32)], axis=1).astype(np.float32))
    in_maps = []
    for i in range(B):
        in_maps.append({
            "x": np.ascontiguousarray(x[i].reshape(C, N)),
            "wqv16": wqv16, "smalls": smalls,
        })
    return in_maps


def kernel(x, Wq, bq, Wk, bk, Wv, bv, _trace=False, _tmpdir=None):
    from concourse.bass_utils import run_bass_kernel_spmd

    x = np.asarray(x, dtype=np.float32)
    B, c, H, W = x.shape
    assert (c, H * W) == (C, N), (c, H, W)
    in_maps = make_in_maps(x, Wq, bq, Wk, bk, Wv, bv)
    nc = _get_nc()
    res = run_bass_kernel_spmd(nc, in_maps, core_ids=list(range(B)),
                               trace=_trace, tmpdir=_tmpdir)
    out = np.stack([res.results[i]["out"].reshape(C, H, W) for i in range(B)])
    if _trace:
        _cache["last_result"] = res
    return out.astype(np.float32)


# revision 47
# speedup vs baseline: 1.2124x; 1.0147x over previous
"""AreaAttention Trainium2 kernel: B=8 data-parallel over 8 NeuronCores.

Reference computation (per sample, C=128 channels, N=H*W=4096 pixels):
    q = Wq@x + bq                    ('oc,bcn->bno' proper matmul)
    k = x * colsum(Wk) + bk          ('oc,bcn->bcn' keeps c: per-channel scale!)
    v = Wv@x + bv                    ('oc,bcn->bno')
    out = x + softmax(q^T k / sqrt(C)) @ v^T

Per-core design (one sample per core, no collectives):
  - q16/k16 stored [c, n] fp16; v16 stored [m, c] fp16 (PSUM accumulates fp32).
  - Scores computed TRANSPOSED: sT[m, n] = k_chunk^T @ q  (free dim 512).
  - exp((s*scale)+E) with global shift E=+1 (cancels in softmax). Most pairs
    on ScalarE (ACT); DVE_PAIRS per block are computed on the Vector engine
    via a Schraudolph bit-trick: fp16_bits = round(A*(s*scale+E)+B) emitted
    by one tensor_scalar into a uint16 view (rms rel err ~1.8%, verified on
    HW). ACT and DVE exps run CONCURRENTLY on different PSUM banks, lifting
    the per-block exp throughput above either engine alone.
  - PV: out[c, n] += v_chunk^T @ expS  -> output directly in [c, n] layout.
  - Softmax denominator: 2 fp16 partial-sum chains over chunk-pairs, ones-
    matmul reduces partitions AND broadcasts row-sums; reciprocal via the
    single-instruction reciprocal_approx_fast (fp32, ~51 ULP), multiply,
    +residual.
  - Setup offload: xf16 casts / k16 scale / residual adds run on the idle
    GpSimd engine; q-projections are spread across blocks (q slice s+1 is
    produced during block s) to unclog DVE in block 0.
"""
import numpy as np

C = 128
N = 4096          # 64*64
NB = 1024         # n-block span
NBLK = N // NB    # 4
MCH = N // C      # 32 m-chunks
NPAIR = MCH // 2  # 16 chunk-pairs per block
SCALE = 1.0 / np.sqrt(np.float32(C))
ESHIFT = 1.0      # global softmax shift: es = exp(s*scale + E); cancels.

# Schraudolph fp16 exp: bits = round(A16*(arg) + B16); bitcast -> fp16 ~ e^arg
A16 = 1477.319722
B16 = 15300.5
DVE_MULT = float(A16 * SCALE)
DVE_ADD = float(A16 * ESHIFT + B16)
# pairs computed on DVE instead of ACT, per block (avoid the hoisted pairs
# {0,1,2} and block-0's setup jps; block 0's DVE is busier with v16/qproj)
DVE_PAIRS = {0: (5, 12), 1: (6, 12), 2: (6, 12), 3: (6, 12)}
# the very last pair: chunk u=0 on ACT, u=1 on DVE, so both exps finish
# together and the final tail starts as early as possible
SPLIT_PAIR = (3, 15)

_cache = {}


def _build_nc():
    import concourse.tile as tile
    from concourse import bacc, mybir

    f32 = mybir.dt.float32
    f16 = mybir.dt.float16
    u16 = mybir.dt.uint16
    ADD = mybir.AluOpType.add
    MUL = mybir.AluOpType.mult
    EXP = mybir.ActivationFunctionType.Exp

    nc = bacc.Bacc("TRN2", target_bir_lowering=False)

    x_d = nc.dram_tensor("x", [C, N], f32, kind="ExternalInput")
    # packed weights: one tiny DMA each instead of six serialized ones
    wqv16_d = nc.dram_tensor("wqv16", [C, 2 * C], f16, kind="ExternalInput")
    smalls_d = nc.dram_tensor("smalls", [C, 4], f32, kind="ExternalInput")
    out_d = nc.dram_tensor("out", [C, N], f32, kind="ExternalOutput")

    with tile.TileContext(nc) as tc:
        with tc.tile_pool(name="big", bufs=1) as big, \
             tc.tile_pool(name="small", bufs=1) as small, \
             tc.tile_pool(name="es_pool", bufs=8) as es_pool, \
             tc.tile_pool(name="p_pool", bufs=6) as p_pool, \
             tc.tile_pool(name="work", bufs=2) as work, \
             tc.tile_pool(name="ps_sc", bufs=3, space="PSUM") as ps_sc, \
             tc.tile_pool(name="ps_pv", bufs=1, space="PSUM") as ps_pv:

            xfb = big.tile([C, N], f32, tag="xfb")      # x, then x + bv (residual)
            xf16 = big.tile([C, N], f16, tag="xf16")    # x fp16 (q/v proj, k build)
            q16 = big.tile([C, N], f16, tag="q16")
            k16 = big.tile([C, N], f16, tag="k16")
            v16 = big.tile([C, N], f16, tag="v16")      # chunk j at cols [128j,128j+128) = v[m, c]

            wqv16 = small.tile([C, 2 * C], f16, tag="wqv16")
            smalls = small.tile([C, 4], f32, tag="smalls")
            ebias = small.tile([C, 1], f32, tag="ebias")
            ones16 = small.tile([C, C], f16, tag="ones16")
            wqt16 = wqv16[:, 0:C]
            wvt16 = wqv16[:, C:2 * C]
            wks = smalls[:, 0:1]
            bk = smalls[:, 1:2]
            bq = smalls[:, 2:3]
            bv = smalls[:, 3:4]

            # Per-queue DMA bandwidth is ~50GB/s, so spread the loads: slice 0
            # split across sync+vector queues (it gates the first QK), the
            # small weights FIRST on the scalar queue (they gate qproj/k16 and
            # would otherwise queue behind megabytes of x), bulk x on gpsimd.
            nc.sync.dma_start(xfb[:, 0:512], x_d[:, 0:512])
            nc.sync.dma_start(smalls[:], smalls_d[:])
            nc.scalar.dma_start(xfb[:, 512:NB], x_d[:, 512:NB])
            nc.scalar.dma_start(wqv16[:], wqv16_d[:])
            nc.sync.dma_start(xfb[:, NB:2 * NB], x_d[:, NB:2 * NB])
            nc.gpsimd.dma_start(xfb[:, 2 * NB:3 * NB], x_d[:, 2 * NB:3 * NB])
            nc.sync.dma_start(xfb[:, 3 * NB:4 * NB], x_d[:, 3 * NB:4 * NB])
            nc.vector.memset(ebias[:], ESHIFT)
            nc.vector.memset(ones16[:], 1.0)

            def cast_k(s):
                """xf16 cast + k16 build for one slice (DVE)."""
                sl = slice(s * NB, (s + 1) * NB)
                nc.vector.tensor_copy(xf16[:, sl], xfb[:, sl])
                nc.vector.tensor_scalar(k16[:, sl], xf16[:, sl], wks, bk,
                                        op0=MUL, op1=ADD)

            def qproj(s):
                """q16 for one 1024-col slice (2 matmuls + DVE bias add)."""
                sl = slice(s * NB, (s + 1) * NB)
                ps = ps_sc.tile([C, NB], f32, tag="sc", name=f"qps{s}")
                for h in range(2):
                    hsl = slice(s * NB + h * 512, s * NB + (h + 1) * 512)
                    nc.tensor.matmul(ps[:, h * 512:(h + 1) * 512], wqt16,
                                     xf16[:, hsl], start=True, stop=True)
                nc.vector.tensor_scalar(q16[:, sl], ps[:], bq, None, op0=ADD)

            def setup_v(s):
                """v16 chunks for one 1024-col slice (8 chunks, one psum tile)."""
                sl = slice(s * NB, (s + 1) * NB)
                psv = ps_sc.tile([C, NB], f32, tag="sc", name=f"vps{s}")
                for t in range(8):
                    j = s * 8 + t
                    nc.tensor.matmul(psv[:, t * C:(t + 1) * C],
                                     xf16[:, j * C:(j + 1) * C], wvt16,
                                     start=True, stop=True)
                nc.vector.tensor_copy(v16[:, sl], psv[:])

            def resid_slice(s):
                sl = slice(s * NB, (s + 1) * NB)
                nc.vector.tensor_scalar(xfb[:, sl], xfb[:, sl], bv, None, op0=ADD)

            def tail(nb, pv, chains, last_es2):
                n0 = nb * NB
                last = nb == NBLK - 1
                # row-sum over partitions; the ones[128,128] stationary operand
                # broadcasts the sum to all partitions. Sources: the two chains
                # (ready one pair early) + the final pair's es2 directly.
                # per-half rs tiles: half 1's rowsum matmuls must not carry a
                # tile-level WAR dependency on half 0's reciprocal read
                if last:
                    rs_t = [ps_sc.tile([C, 512], f32, tag="sc", name=f"rs{nb}_{h}")
                            for h in range(2)]
                else:
                    rs1 = ps_sc.tile([C, NB], f32, tag="sc", name=f"rs{nb}")
                    rs_t = [rs1[:, 0:512], rs1[:, 512:NB]]
                srcs = [(chains[0], slice(0, NB)), (chains[1], slice(0, NB))]
                if last_es2 is not None:
                    srcs.append((last_es2, slice(0, NB)))
                # single-instruction approximate reciprocal (fp32, ~51 ULP),
                # then normalize + residual and DMA out; in the final block
                # the rowsums are inside the half loop so half 0's normalize
                # overlaps half 1's rowsum matmuls.
                rb = work.tile([C, NB], f32, tag="rb", name=f"rb{nb}")
                ep1 = work.tile([C, NB], f32, tag="ep1", name=f"ep{nb}")
                ost = work.tile([C, NB], f32, tag="ost", name=f"ost{nb}")
                halves = (slice(0, 512), slice(512, NB)) if last else (slice(0, NB),)
                for hs in halves:
                    for h in (0, 1) if hs.stop - hs.start == NB else (hs.start // 512,):
                        hsl = slice(h * 512, (h + 1) * 512)
                        for ci, (src, _) in enumerate(srcs):
                            dst = rs_t[h][:, 0:512] if last else rs1[:, hsl]
                            nc.tensor.matmul(dst, ones16[:], src[:, hsl],
                                             start=(ci == 0), stop=False)
                            nc.tensor.matmul(dst, ones16[:],
                                             src[:, NB + h * 512:NB + (h + 1) * 512],
                                             start=False, stop=(ci == len(srcs) - 1))
                    osl = slice(n0 + hs.start, n0 + hs.stop)
                    if last:
                        rsrc = rs_t[hs.start // 512][:, 0:512]
                    else:
                        rsrc = rs1[:, hs]
                    nc.vector.reciprocal_approx_fast(out=rb[:, hs], in_=rsrc)
                    nc.vector.tensor_tensor(ep1[:, hs], pv[:, hs], rb[:, hs], op=MUL)
                    nc.vector.tensor_tensor(ost[:, hs], ep1[:, hs], xfb[:, osl], op=ADD)
                    if last:
                        # split across two queues so the final DMA is small
                        mid = (hs.start + hs.stop) // 2
                        nc.sync.dma_start(out_d[:, n0 + hs.start:n0 + mid],
                                          ost[:, hs.start:mid])
                        nc.scalar.dma_start(out_d[:, n0 + mid:n0 + hs.stop],
                                            ost[:, mid:hs.stop])
                    else:
                        nc.sync.dma_start(out_d[:, osl], ost[:, hs])

            # slice-0 setup at half granularity so the first QK/exp is not
            # gated on the second x-half DMA; later slices' casts are
            # interleaved into block 0 (setup_v needs xf16 of every slice).
            for h in range(2):
                hsl = slice(h * 512, (h + 1) * 512)
                qps0 = ps_sc.tile([C, 512], f32, tag="sc", name=f"qps0_{h}")
                nc.vector.tensor_copy(xf16[:, hsl], xfb[:, hsl])
                nc.vector.tensor_scalar(k16[:, hsl], xf16[:, hsl], wks, bk,
                                        op0=MUL, op1=ADD)
                nc.tensor.matmul(qps0[:, 0:512], wqt16, xf16[:, hsl],
                                 start=True, stop=True)
                nc.vector.tensor_scalar(q16[:, hsl], qps0[:, 0:512], bq, None,
                                        op0=ADD)

            def emit_qk_exp(nb, jp):
                """QK matmuls + exp for one chunk-pair; returns the es2 tile.

                ACT pairs: scalar.activation Exp (exact). DVE_PAIRS: one
                vector.tensor_scalar per chunk writing Schraudolph fp16 bits
                through a uint16 view (runs concurrently with ACT pairs).
                """
                n0 = nb * NB
                es2 = es_pool.tile([C, 2 * NB], f16, tag="es", name=f"es{nb}_{jp}")
                for u in range(2):
                    on_dve = (jp in DVE_PAIRS[nb]) or ((nb, jp) == SPLIT_PAIR and u == 1)
                    j = 2 * jp + u
                    ksl = slice(j * C, (j + 1) * C)
                    sc = ps_sc.tile([C, NB], f32, tag="sc", name=f"sc{nb}_{j}")
                    for h in range(2):
                        nc.tensor.matmul(sc[:, h * 512:(h + 1) * 512],
                                         k16[:, ksl],
                                         q16[:, n0 + h * 512:n0 + (h + 1) * 512],
                                         start=True, stop=True)
                    usl = slice(u * NB, (u + 1) * NB)
                    if on_dve:
                        nc.vector.tensor_scalar(es2[:, usl].bitcast(u16), sc[:],
                                                DVE_MULT, DVE_ADD,
                                                op0=MUL, op1=ADD)
                    elif nb == 0 and jp == 0:
                        # very first pair: per-512 activations so the first
                        # exp starts before the second QK half lands
                        for h in range(2):
                            ql = slice(u * NB + h * 512, u * NB + (h + 1) * 512)
                            nc.scalar.activation(es2[:, ql],
                                                 sc[:, h * 512:(h + 1) * 512],
                                                 EXP, bias=ebias[:],
                                                 scale=float(SCALE))
                    else:
                        nc.scalar.activation(es2[:, usl], sc[:], EXP,
                                             bias=ebias[:], scale=float(SCALE))
                return es2

            def emit_pv(nb, jp, pv, es2):
                for u in range(2):
                    j = 2 * jp + u
                    ksl = slice(j * C, (j + 1) * C)
                    for h in range(2):
                        nc.tensor.matmul(pv[:, h * 512:(h + 1) * 512],
                                         v16[:, ksl],
                                         es2[:, u * NB + h * 512:u * NB + (h + 1) * 512],
                                         start=(j == 0), stop=(j == MCH - 1))

            NHOIST = 3  # next-block QK/exp pairs emitted ahead of each tail
            hoisted = {}
            for nb in range(NBLK):
                pv = ps_pv.tile([C, NB], f32, tag="pv", name=f"pv{nb}")
                chains = [p_pool.tile([C, 2 * NB], f16, tag="pacc", name=f"pacc{nb}_{i}")
                          for i in range(2)]
                pend = None  # deferred chain op: DVE exps jump ahead of it

                def flush_chain():
                    nonlocal pend
                    if pend is None:
                        return
                    jq, es = pend
                    pend = None
                    if jq < 2:
                        nc.vector.tensor_copy(chains[jq % 2][:], es[:])
                    else:
                        nc.vector.tensor_tensor(chains[jq % 2][:],
                                                chains[jq % 2][:], es[:], op=ADD)

                for jp in range(NPAIR):
                    if nb == 0 and jp in (3, 7, 11):
                        s = (jp + 1) // 4       # slices 1, 2, 3
                        cast_k(s)
                        setup_v(s)
                    if jp == 5 and nb < NBLK - 1:
                        qproj(nb + 1)           # q for the NEXT block
                    if jp == 9:
                        resid_slice(nb)         # xfb += bv for this block
                    es2 = hoisted.pop((nb, jp), None)
                    if es2 is None:
                        es2 = emit_qk_exp(nb, jp)
                    flush_chain()               # previous pair's chain op
                    if nb == 0 and jp == 0:
                        setup_v(0)
                    # hoist the NEXT block's first QK/exp pairs ahead of this
                    # block's final PVs + tail so ACT never waits at the boundary
                    if jp == NPAIR - 1 and nb < NBLK - 1:
                        for hj in range(NHOIST):
                            hoisted[(nb + 1, hj)] = emit_qk_exp(nb + 1, hj)
                    emit_pv(nb, jp, pv, es2)
                    # denominator: 2 interleaved fp16 chains over chunk-pairs,
                    # deferred one pair so a DVE-pair exp runs before it. In
                    # the FINAL block the last pair skips the chain -- its
                    # contribution goes straight into the rowsum matmuls.
                    if jp == NPAIR - 1 and nb == NBLK - 1:
                        last_es2 = es2
                    else:
                        pend = (jp, es2)

                flush_chain()
                tail(nb, pv, chains, last_es2 if nb == NBLK - 1 else None)

    nc.finalize()
    return nc


def _get_nc():
    if "nc" not in _cache:
        _cache["nc"] = _build_nc()
    return _cache["nc"]


def make_in_maps(x, Wq, bq, Wk, bk, Wv, bv):
    x = np.asarray(x, dtype=np.float32)
    B = x.shape[0]
    wqt16 = np.asarray(Wq, np.float32).T.astype(np.float16)
    wvt16 = np.asarray(Wv, np.float32).T.astype(np.float16)
    wqv16 = np.ascontiguousarray(np.concatenate([wqt16, wvt16], axis=1))
    wks = np.asarray(Wk, np.float32).sum(axis=0)
    smalls = np.ascontiguousarray(np.stack(
        [wks, np.asarray(bk, np.float32), np.asarray(bq, np.float32),
         np.asarray(bv, np.float32)], axis=1).astype(np.float32))
    in_maps = []
    for i in range(B):
        in_maps.append({
            "x": np.ascontiguousarray(x[i].reshape(C, N)),
            "wqv16": wqv16, "smalls": smalls,
        })
    return in_maps


def kernel(x, Wq, bq, Wk, bk, Wv, bv, _trace=False, _tmpdir=None):
    from concourse.bass_utils import run_bass_kernel_spmd

    x = np.asarray(x, dtype=np.float32)
    B, c, H, W = x.shape
    assert (c, H * W) == (C, N), (c, H, W)
    in_maps = make_in_maps(x, Wq, bq, Wk, bk, Wv, bv)
    nc = _get_nc()
    res = run_bass_kernel_spmd(nc, in_maps, core_ids=list(range(B)),
                               trace=_trace, tmpdir=_tmpdir)
    out = np.stack([res.results[i]["out"].reshape(C, H, W) for i in range(B)])
    if _trace:
        _cache["last_result"] = res
    return out.astype(np.float32)
